# revision 15
# baseline (speedup 1.0000x reference)
"""Nadaraya-Watson head (retrieval kNN) Trainium2 Bass kernel — v4.

reference:
    dist = ||q - x||_2 over d            (b, s)
    probs = softmax(-dist, axis=s)       (b, s)
    out = probs @ labels                 (b, c)

Strategy (8 NeuronCores, batch-parallel, 8 batches per core):
  X is shipped in TRANSPOSED fp8 layout XT[b, ki, h, s] = X[b, s, 128h+ki]
  (d on partitions, two 128-halves). This enables:
    - ONE fused ACT pass per d-half: W2 = Square(x * 1 + (-q_d)) with the
      query entering through the per-partition activation bias -> (x - q)^2
      directly from fp8 input, bf16 out. No DVE subtract, no fold tree.
    - dist^2 = sum_d (x-q)^2 via PE matmuls: stationary = W2 chunk
      [128(d-half), 128(s-cols)] (128-col bf16 -> FWL fast weight load),
      moving = ones [128, 1]; the two d-halves accumulate in PSUM, giving
      dist^2 columns [128, 64] per batch directly in stats-friendly layout:
      score (col, part) <-> support row s = 128*col + part.
  fp8 X halves HBM traffic (the memory-bound term); q stays fp32 exact in
  the bias so only x carries quantization noise (~0.02 logit std, harmless
  under the diffuse softmax).
  dist = sqrt via linear seed + 2 Newton-Raphson steps on DVE (avoids the
  sqrt ACT table set; Square/Exp share the exp_and_others set -> one table
  load total).
  Softmax shift is a constant (exact math; dist concentrates near 22.6 so
  exp stays in range without a max pass).
  Label reduction: PE matmul with fp8 labels stationary [128, 104] (cols
  0-99 labels, 100 = ones column giving the normalizer Z, 101-103 pad) and
  the fp8 probs column [128, 1] moving, PSUM rotating over 4 banks. The
  labels are host-repacked so stationary chunk `col` holds rows
  s = 128*col + part, matching the score layout. Host divides by Z.
"""

from contextlib import ExitStack

import ml_dtypes
import numpy as np

import concourse.bacc as bacc
import concourse.tile as tile
from concourse import mybir
from concourse.bass_utils import run_bass_kernel_spmd

F32 = mybir.dt.float32
BF16 = mybir.dt.bfloat16
FP8 = mybir.dt.float8e4
OP = mybir.AluOpType
AF = mybir.ActivationFunctionType

# Problem sizes (hardcoded per harness contract).
B, S, D, C = 64, 8192, 256, 100
CA = 104                   # labels + ones col + pad to 8B multiple
NCORES = 8
BPC = B // NCORES          # batches per core
NH = 2                     # d split into two 128-partition halves

# Constant softmax shift: exp(SHIFT - dist). Exact math (softmax is
# shift-invariant); dist concentrates near sqrt(2*D) ~ 22.6.
SHIFT = 22.0

# Minimax linear seed for sqrt(v) on v in [250, 900] (dist^2 range with huge
# margin), refined by two Newton-Raphson steps -> rel err ~1e-7.
FIT_B = 0.0218287
FIT_A = 10.9031

KPACKL = 64                # label score-columns packed per L DMA tile


def _build_nc(bpc=BPC, s=S, dve_cols=20, kpackl=KPACKL):
    """nt: X tiles per batch (s-slices). kpackl: score columns of labels
    packed per L DMA tile."""
    nchunk = s // 128         # score columns per batch
    sd = dve_cols * 128       # s-extent handled by DVE (tail of each batch)
    sa = s - sd               # s-extent handled by ACT
    nblkl = nchunk // kpackl
    nc = bacc.Bacc(None)
    XT = nc.declare_dram_parameter("xt", [bpc, 128, NH, s], FP8, isOutput=False)
    QN = nc.declare_dram_parameter("qn", [bpc, 128, NH], F32, isOutput=False)
    # L repacked on host: l[b, j, part, a*CA + c] = L[b, 128*(kpackl*j + a) + part, c]
    L = nc.declare_dram_parameter(
        "l", [bpc, nblkl, 128, kpackl * CA], FP8, isOutput=False
    )
    # out[:, 0:100] = unnormalized label sums, out[:, 100] = Z; host divides.
    OUT = nc.declare_dram_parameter("out", [bpc, CA], F32, isOutput=True)

    with tile.TileContext(nc) as tc, ExitStack() as ctx:
        xpool = ctx.enter_context(tc.tile_pool(name="xpool", bufs=3))
        wpool = ctx.enter_context(tc.tile_pool(name="wpool", bufs=2))
        lpool = ctx.enter_context(tc.tile_pool(name="lpool", bufs=4))
        qpool = ctx.enter_context(tc.tile_pool(name="qpool", bufs=3))
        stats = ctx.enter_context(tc.tile_pool(name="stats", bufs=2))
        outp = ctx.enter_context(tc.tile_pool(name="outp", bufs=2))
        cons = ctx.enter_context(tc.tile_pool(name="cons", bufs=1))
        psumv = ctx.enter_context(tc.tile_pool(name="psumv", bufs=3, space="PSUM"))
        psuma = ctx.enter_context(tc.tile_pool(name="psuma", bufs=1, space="PSUM"))

        shiftt = cons.tile([128, 1], F32)
        nc.vector.memset(shiftt[:], SHIFT)
        warm = cons.tile([128, 1], F32)
        nc.scalar.activation(out=warm[:], in_=shiftt[:], func=AF.Square)
        ones1 = cons.tile([128, 1], BF16)
        nc.vector.memset(ones1[:], 1.0)

        qnts = {}
        vpss = {}
        ps = {}

        xts = {}
        wsqs = {}
        pts = {}

        def emit_dmas(b):
            qnt = qpool.tile([128, NH], F32, tag="qn")
            nc.sync.dma_start(qnt[:], QN[b])
            qnts[b] = qnt
            xt = xpool.tile([128, NH, s], FP8, tag="xt")
            for h in range(NH):
                nc.sync.dma_start(xt[:, h], XT[b][:, h])
            xts[b] = xt
            lts = []
            for j in range(nblkl):
                lt = lpool.tile([128, kpackl * CA], FP8, tag=f"lt{j}")
                nc.sync.dma_start(lt[:], L[b][j])
                lts.append(lt)
            ps[b] = lts

        def emit_B(b):
            # NR sqrt chain on DVE; emitted early so EXP(b) becomes ready
            # while SQ(b+1) still runs on ACT.
            vps = vpss[b]
            y0 = stats.tile([128, nchunk], F32, tag="y0")
            nc.vector.tensor_scalar(
                out=y0[:], in0=vps[:], scalar1=FIT_B, scalar2=FIT_A,
                op0=OP.mult, op1=OP.add,
            )
            v = stats.tile([128, nchunk], F32, tag="v")
            nc.vector.tensor_copy(v[:], vps[:])
            ycur = y0
            for it in range(2):
                r = stats.tile([128, nchunk], F32, tag=f"r{it}")
                nc.vector.reciprocal(r[:], ycur[:])
                t_ = stats.tile([128, nchunk], F32, tag=f"t{it}")
                nc.vector.tensor_mul(t_[:], v[:], r[:])
                u = stats.tile([128, nchunk], F32, tag=f"u{it}")
                nc.vector.tensor_add(u[:], ycur[:], t_[:])
                ynext = stats.tile([128, nchunk], F32, tag=f"y{it + 1}")
                nc.vector.tensor_scalar(
                    out=ynext[:], in0=u[:], scalar1=0.5, scalar2=None,
                    op0=OP.mult,
                )
                ycur = ynext
            pts[b] = ycur

        def emit_SQ(b):
            qnt = qnts[b]
            xt = xts[b]
            wsq = wpool.tile([128, NH, s], BF16, tag="wsq")
            wsqs[b] = wsq
            for h in range(NH):
                nc.scalar.activation(
                    out=wsq[:, h, 0:sa], in_=xt[:, h, 0:sa], func=AF.Square,
                    scale=1.0, bias=qnt[:, h:h + 1],
                )

        def emit_DVEcols(b):
            qnt = qnts[b]
            xt = xts[b]
            wsq = wsqs[b]
            wt = wpool.tile([128, NH, sd], BF16, tag="wt")
            for h in range(NH):
                nc.vector.tensor_scalar(
                    out=wt[:, h], in0=xt[:, h, sa:s], scalar1=qnt[:, h:h + 1],
                    scalar2=None, op0=OP.add,
                )
            nc.vector.tensor_mul(wsq[:, :, sa:s], wt[:], wt[:])

        def emit_EXP(b):
            ycur = pts.pop(b)
            p = stats.tile([128, nchunk], FP8, tag="p")
            nc.scalar.activation(
                out=p[:], in_=ycur[:], func=AF.Exp, scale=-1.0, bias=shiftt[:],
            )
            pts[(b, "p")] = p

        def emit_scores(b):
            wsq = wsqs.pop(b)
            vps = psumv.tile([128, nchunk], F32, tag="vps", name=f"vps{b}")
            vpss[b] = vps
            for col in range(nchunk):
                for h in range(NH):
                    nc.tensor.matmul(
                        vps[:, col:col + 1],
                        wsq[:, h, col * 128:(col + 1) * 128],
                        ones1[:],
                        start=(h == 0), stop=(h == NH - 1),
                    )

        def emit_C(b):
            vpss.pop(b)
            p = pts.pop((b, "p"))
            lts = ps.pop(b)
            NBANK = 4
            accs = [
                psuma.tile([CA, 1], F32, tag=f"acc{g}", name=f"acc{g}_{b}")
                for g in range(NBANK)
            ]
            for j in range(nblkl):
                lt = lts[j]
                for a in range(kpackl):
                    col = j * kpackl + a
                    nc.tensor.matmul(
                        accs[col % NBANK][:],
                        lt[:, a * CA:(a + 1) * CA],
                        p[:, col:col + 1],
                        start=(col < NBANK), stop=(col >= nchunk - NBANK),
                    )
            c0 = outp.tile([CA, 1], F32, tag="c0")
            nc.vector.tensor_copy(c0[:], accs[0][:])
            c1 = outp.tile([CA, 1], F32, tag="c1")
            nc.vector.tensor_add(c1[:], c0[:], accs[1][:])
            c2 = outp.tile([CA, 1], F32, tag="c2")
            nc.vector.tensor_add(c2[:], c1[:], accs[2][:])
            stot = outp.tile([CA, 1], F32, tag="stot")
            nc.vector.tensor_add(stot[:], c2[:], accs[3][:])
            nc.gpsimd.dma_start(OUT[b], stot[:, 0])

        for b in range(bpc):
            emit_dmas(b)
            emit_SQ(b)
            emit_DVEcols(b)
            emit_scores(b)
            if b >= 1:
                emit_B(b - 1)
                emit_EXP(b - 1)
                emit_C(b - 1)
        emit_B(bpc - 1)
        emit_EXP(bpc - 1)
        emit_C(bpc - 1)


    nc.finalize()
    return nc


_NC_CACHE = []
LAST_RESULT = None
BF = ml_dtypes.bfloat16
F8 = ml_dtypes.float8_e4m3


def _prep_core(q, X, L):
    """Host-side prep for one core's slice: dtype casts + layout only.
    XT[b, ki, h, s] = X[b, s, 128h+ki] in fp8; qn = -q in the matching
    [128, 2] per-partition layout (fp32, exact); L padded to 104 cols with
    a ones column at 100 and repacked so the stationary chunk for score
    column `col` holds rows s = 128*col + part."""
    bpc, s, d = X.shape
    nchunk = s // 128
    nblkl = nchunk // KPACKL
    xt = np.ascontiguousarray(
        X.transpose(0, 2, 1).reshape(bpc, NH, 128, s).transpose(0, 2, 1, 3)
    ).astype(F8)
    qn = (-q.astype(np.float32)).reshape(bpc, NH, 128).transpose(0, 2, 1)
    qn = np.ascontiguousarray(qn)
    Laug = np.zeros((bpc, s, CA), dtype=F8)
    Laug[:, :, :C] = L
    Laug[:, :, C] = 1.0
    # [b, s, c] with s = 128*(kpackl*j + a) + part -> [b, j, part, a, c]
    Lr = Laug.reshape(bpc, nblkl, KPACKL, 128, CA).transpose(0, 1, 3, 2, 4)
    Lr = np.ascontiguousarray(Lr).reshape(bpc, nblkl, 128, KPACKL * CA)
    return {"xt": xt, "qn": qn, "l": Lr}


def kernel(**inputs) -> np.ndarray:
    global LAST_RESULT
    q = np.asarray(inputs["query_feats"], dtype=np.float32)
    X = np.asarray(inputs["support_feats"], dtype=np.float32)
    L = np.asarray(inputs["support_labels"], dtype=np.float32)
    assert q.shape == (B, D) and X.shape == (B, S, D) and L.shape == (B, S, C)

    if not _NC_CACHE:
        _NC_CACHE.append(_build_nc())
    nc = _NC_CACHE[0]

    in_maps = []
    for c in range(NCORES):
        sl = slice(c * BPC, (c + 1) * BPC)
        in_maps.append(_prep_core(q[sl], X[sl], L[sl]))

    res = run_bass_kernel_spmd(nc, in_maps, list(range(NCORES)))
    LAST_RESULT = res
    raw = np.concatenate([res.results[c]["out"] for c in range(NCORES)], axis=0)
    out = raw[:, :C] / raw[:, C:C + 1]
    return out.astype(np.float32)


# revision 16
# speedup vs baseline: 1.1071x; 1.1071x over previous
"""Nadaraya-Watson head (retrieval kNN) Trainium2 Bass kernel — v4.

reference:
    dist = ||q - x||_2 over d            (b, s)
    probs = softmax(-dist, axis=s)       (b, s)
    out = probs @ labels                 (b, c)

Strategy (8 NeuronCores, batch-parallel, 8 batches per core):
  X is shipped in TRANSPOSED fp8 layout XT[b, ki, h, s] = X[b, s, 128h+ki]
  (d on partitions, two 128-halves). This enables:
    - ONE fused ACT pass per d-half: W2 = Square(x * 1 + (-q_d)) with the
      query entering through the per-partition activation bias -> (x - q)^2
      directly from fp8 input, bf16 out. No DVE subtract, no fold tree.
    - dist^2 = sum_d (x-q)^2 via PE matmuls: stationary = W2 chunk
      [128(d-half), 128(s-cols)] (128-col bf16 -> FWL fast weight load),
      moving = ones [128, 1]; the two d-halves accumulate in PSUM, giving
      dist^2 columns [128, 64] per batch directly in stats-friendly layout:
      score (col, part) <-> support row s = 128*col + part.
  fp8 X halves HBM traffic (the memory-bound term); q stays fp32 exact in
  the bias so only x carries quantization noise (~0.02 logit std, harmless
  under the diffuse softmax).
  dist = sqrt via linear seed + 2 Newton-Raphson steps on DVE (avoids the
  sqrt ACT table set; Square/Exp share the exp_and_others set -> one table
  load total).
  Softmax shift is a constant (exact math; dist concentrates near 22.6 so
  exp stays in range without a max pass).
  Label reduction: PE matmul with fp8 labels stationary [128, 104] (cols
  0-99 labels, 100 = ones column giving the normalizer Z, 101-103 pad) and
  the fp8 probs column [128, 1] moving, PSUM rotating over 4 banks. The
  labels are host-repacked so stationary chunk `col` holds rows
  s = 128*col + part, matching the score layout. Host divides by Z.
"""

from contextlib import ExitStack

import ml_dtypes
import numpy as np

import concourse.bacc as bacc
import concourse.tile as tile
from concourse import mybir
from concourse.bass_utils import run_bass_kernel_spmd

F32 = mybir.dt.float32
BF16 = mybir.dt.bfloat16
FP8 = mybir.dt.float8e4
OP = mybir.AluOpType
AF = mybir.ActivationFunctionType

# Problem sizes (hardcoded per harness contract).
B, S, D, C = 64, 8192, 256, 100
CA = 104                   # labels + ones col + pad to 8B multiple
NCORES = 8
BPC = B // NCORES          # batches per core
NH = 2                     # d split into two 128-partition halves

# Constant softmax shift: exp(SHIFT - dist). Exact math (softmax is
# shift-invariant); dist concentrates near sqrt(2*D) ~ 22.6.
SHIFT = 22.0

# Minimax linear seed for sqrt(v) on v in [250, 900] (dist^2 range with huge
# margin), refined by two Newton-Raphson steps -> rel err ~1e-7.
FIT_B = 0.0218287
FIT_A = 10.9031

KPACKL = 64                # label score-columns packed per L DMA tile


def _build_nc(bpc=BPC, s=S, dve_cols=20, kpackl=KPACKL):
    """nt: X tiles per batch (s-slices). kpackl: score columns of labels
    packed per L DMA tile."""
    nchunk = s // 128         # score columns per batch
    sd = dve_cols * 128       # s-extent handled by DVE (tail of each batch)
    sa = s - sd               # s-extent handled by ACT
    nblkl = nchunk // kpackl
    nc = bacc.Bacc(None)
    XT = nc.declare_dram_parameter("xt", [bpc, 128, NH, s], FP8, isOutput=False)
    QN = nc.declare_dram_parameter("qn", [bpc, 128, NH], F32, isOutput=False)
    # L repacked on host: l[b, j, part, a*CA + c] = L[b, 128*(kpackl*j + a) + part, c]
    L = nc.declare_dram_parameter(
        "l", [bpc, nblkl, 128, kpackl * CA], FP8, isOutput=False
    )
    # out[:, 0:100] = unnormalized label sums, out[:, 100] = Z; host divides.
    OUT = nc.declare_dram_parameter("out", [bpc, CA], F32, isOutput=True)

    with tile.TileContext(nc) as tc, ExitStack() as ctx:
        xpool = ctx.enter_context(tc.tile_pool(name="xpool", bufs=4))
        wpool = ctx.enter_context(tc.tile_pool(name="wpool", bufs=2))
        lpool = ctx.enter_context(tc.tile_pool(name="lpool", bufs=4))
        qpool = ctx.enter_context(tc.tile_pool(name="qpool", bufs=3))
        stats = ctx.enter_context(tc.tile_pool(name="stats", bufs=2))
        outp = ctx.enter_context(tc.tile_pool(name="outp", bufs=2))
        cons = ctx.enter_context(tc.tile_pool(name="cons", bufs=1))
        psumv = ctx.enter_context(tc.tile_pool(name="psumv", bufs=3, space="PSUM"))
        psuma = ctx.enter_context(tc.tile_pool(name="psuma", bufs=1, space="PSUM"))

        shiftt = cons.tile([128, 1], F32)
        nc.vector.memset(shiftt[:], SHIFT)
        warm = cons.tile([128, 1], F32)
        nc.scalar.activation(out=warm[:], in_=shiftt[:], func=AF.Square)
        ones1 = cons.tile([128, 1], BF16)
        nc.vector.memset(ones1[:], 1.0)

        qnts = {}
        vpss = {}
        ps = {}

        xts = {}
        wsqs = {}
        pts = {}

        def emit_dmas(b):
            qnt = qpool.tile([128, NH], F32, tag="qn")
            nc.sync.dma_start(qnt[:], QN[b])
            qnts[b] = qnt
            xt = xpool.tile([128, NH, s], FP8, tag="xt")
            for h in range(NH):
                nc.sync.dma_start(xt[:, h], XT[b][:, h])
            xts[b] = xt
            lts = []
            for j in range(nblkl):
                lt = lpool.tile([128, kpackl * CA], FP8, tag=f"lt{j}")
                nc.sync.dma_start(lt[:], L[b][j])
                lts.append(lt)
            ps[b] = lts

        def emit_B(b):
            # NR sqrt chain on DVE; emitted early so EXP(b) becomes ready
            # while SQ(b+1) still runs on ACT.
            vps = vpss[b]
            y0 = stats.tile([128, nchunk], F32, tag="y0")
            nc.vector.tensor_scalar(
                out=y0[:], in0=vps[:], scalar1=FIT_B, scalar2=FIT_A,
                op0=OP.mult, op1=OP.add,
            )
            v = stats.tile([128, nchunk], F32, tag="v")
            nc.vector.tensor_copy(v[:], vps[:])
            ycur = y0
            for it in range(2):
                r = stats.tile([128, nchunk], F32, tag=f"r{it}")
                nc.vector.reciprocal(r[:], ycur[:])
                t_ = stats.tile([128, nchunk], F32, tag=f"t{it}")
                nc.vector.tensor_mul(t_[:], v[:], r[:])
                u = stats.tile([128, nchunk], F32, tag=f"u{it}")
                nc.vector.tensor_add(u[:], ycur[:], t_[:])
                ynext = stats.tile([128, nchunk], F32, tag=f"y{it + 1}")
                nc.vector.tensor_scalar(
                    out=ynext[:], in0=u[:], scalar1=0.5, scalar2=None,
                    op0=OP.mult,
                )
                ycur = ynext
            pts[b] = ycur

        def emit_SQ(b):
            qnt = qnts[b]
            xt = xts[b]
            wsq = wpool.tile([128, NH, s], BF16, tag="wsq")
            wsqs[b] = wsq
            for h in range(NH):
                nc.scalar.activation(
                    out=wsq[:, h, 0:sa], in_=xt[:, h, 0:sa], func=AF.Square,
                    scale=1.0, bias=qnt[:, h:h + 1],
                )

        def emit_DVEcols(b):
            qnt = qnts[b]
            xt = xts[b]
            wsq = wsqs[b]
            wt = wpool.tile([128, NH, sd], BF16, tag="wt")
            for h in range(NH):
                nc.vector.tensor_scalar(
                    out=wt[:, h], in0=xt[:, h, sa:s], scalar1=qnt[:, h:h + 1],
                    scalar2=None, op0=OP.add,
                )
            nc.vector.tensor_mul(wsq[:, :, sa:s], wt[:], wt[:])

        def emit_EXP(b):
            ycur = pts.pop(b)
            p = stats.tile([128, nchunk], FP8, tag="p")
            nc.scalar.activation(
                out=p[:], in_=ycur[:], func=AF.Exp, scale=-1.0, bias=shiftt[:],
            )
            pts[(b, "p")] = p

        def emit_scores(b):
            wsq = wsqs.pop(b)
            vps = psumv.tile([128, nchunk], F32, tag="vps", name=f"vps{b}")
            vpss[b] = vps
            for col in range(nchunk):
                for h in range(NH):
                    nc.tensor.matmul(
                        vps[:, col:col + 1],
                        wsq[:, h, col * 128:(col + 1) * 128],
                        ones1[:],
                        start=(h == 0), stop=(h == NH - 1),
                    )

        def emit_C(b):
            vpss.pop(b)
            p = pts.pop((b, "p"))
            lts = ps.pop(b)
            NBANK = 4
            accs = [
                psuma.tile([CA, 1], F32, tag=f"acc{g}", name=f"acc{g}_{b}")
                for g in range(NBANK)
            ]
            for j in range(nblkl):
                lt = lts[j]
                for a in range(kpackl):
                    col = j * kpackl + a
                    nc.tensor.matmul(
                        accs[col % NBANK][:],
                        lt[:, a * CA:(a + 1) * CA],
                        p[:, col:col + 1],
                        start=(col < NBANK), stop=(col >= nchunk - NBANK),
                    )
            c0 = outp.tile([CA, 1], F32, tag="c0")
            nc.vector.tensor_copy(c0[:], accs[0][:])
            c1 = outp.tile([CA, 1], F32, tag="c1")
            nc.vector.tensor_add(c1[:], c0[:], accs[1][:])
            c2 = outp.tile([CA, 1], F32, tag="c2")
            nc.vector.tensor_add(c2[:], c1[:], accs[2][:])
            stot = outp.tile([CA, 1], F32, tag="stot")
            nc.vector.tensor_add(stot[:], c2[:], accs[3][:])
            nc.gpsimd.dma_start(OUT[b], stot[:, 0])

        for b in range(bpc):
            emit_dmas(b)
            emit_SQ(b)
            emit_DVEcols(b)
            emit_scores(b)
            if b >= 1:
                emit_B(b - 1)
                emit_EXP(b - 1)
                emit_C(b - 1)
        emit_B(bpc - 1)
        emit_EXP(bpc - 1)
        emit_C(bpc - 1)


    nc.finalize()
    return nc


_NC_CACHE = []
LAST_RESULT = None
BF = ml_dtypes.bfloat16
F8 = ml_dtypes.float8_e4m3


def _prep_core(q, X, L):
    """Host-side prep for one core's slice: dtype casts + layout only.
    XT[b, ki, h, s] = X[b, s, 128h+ki] in fp8; qn = -q in the matching
    [128, 2] per-partition layout (fp32, exact); L padded to 104 cols with
    a ones column at 100 and repacked so the stationary chunk for score
    column `col` holds rows s = 128*col + part."""
    bpc, s, d = X.shape
    nchunk = s // 128
    nblkl = nchunk // KPACKL
    xt = np.ascontiguousarray(
        X.transpose(0, 2, 1).reshape(bpc, NH, 128, s).transpose(0, 2, 1, 3)
    ).astype(F8)
    qn = (-q.astype(np.float32)).reshape(bpc, NH, 128).transpose(0, 2, 1)
    qn = np.ascontiguousarray(qn)
    Laug = np.zeros((bpc, s, CA), dtype=F8)
    Laug[:, :, :C] = L
    Laug[:, :, C] = 1.0
    # [b, s, c] with s = 128*(kpackl*j + a) + part -> [b, j, part, a, c]
    Lr = Laug.reshape(bpc, nblkl, KPACKL, 128, CA).transpose(0, 1, 3, 2, 4)
    Lr = np.ascontiguousarray(Lr).reshape(bpc, nblkl, 128, KPACKL * CA)
    return {"xt": xt, "qn": qn, "l": Lr}


def kernel(**inputs) -> np.ndarray:
    global LAST_RESULT
    q = np.asarray(inputs["query_feats"], dtype=np.float32)
    X = np.asarray(inputs["support_feats"], dtype=np.float32)
    L = np.asarray(inputs["support_labels"], dtype=np.float32)
    assert q.shape == (B, D) and X.shape == (B, S, D) and L.shape == (B, S, C)

    if not _NC_CACHE:
        _NC_CACHE.append(_build_nc())
    nc = _NC_CACHE[0]

    in_maps = []
    for c in range(NCORES):
        sl = slice(c * BPC, (c + 1) * BPC)
        in_maps.append(_prep_core(q[sl], X[sl], L[sl]))

    res = run_bass_kernel_spmd(nc, in_maps, list(range(NCORES)))
    LAST_RESULT = res
    raw = np.concatenate([res.results[c]["out"] for c in range(NCORES)], axis=0)
    out = raw[:, :C] / raw[:, C:C + 1]
    return out.astype(np.float32)
